# revision 78
# baseline (speedup 1.0000x reference)
"""DensePatchAttention Trainium2 kernel (v2).

Full (unsharded) inputs -> full output. Internally shards across 8
NeuronCores as (batch b in 0..3) x (head-group g in 0..1, 4 heads each).

Reference computation (per batch):
  q = 1x1conv(x, Wq) + bq                  [256, 128, 128]
  k = 8x8/s8conv(x, Wk) + bk               [256, 16, 16]
  v = 8x8/s8conv(x, Wv) + bv               [256, 16, 16]
  per head h (c=32 channels, channel = c_idx*8 + h):
    dots = q_h^T k_h        [HW=16384, K=256]
    attn = softmax(dots)
    out_h = attn @ v_h      [16384, 32]

v2 design (cost-model-balanced; correctness verified on HW via PJRT):
  - DMA split over three queues (SP / Act / Pool-SWDGE): x streamed as 16
    half-chunks (all of ck0 first so the k conv starts ~7us in), then wk
    interleaved with x-ck1, then wv; output stores ride SP. All large
    transfers have >=512B contiguous runs per partition.
  - PE warm-up dummy matmul chain during the x load (keeps the p-state
    ramp warm so the conv matmuls run at 2.4 GHz sooner).
  - q conv in 2-pt units outside the steady pace: 4 matmuls into one
    [128,1024] psum + ONE bias-copy (Identity+bias on Act for every 3rd
    unit, tensor_scalar_add on DVE otherwise) -> q_t bf16. A matmuls run
    bf16 x bf16 (dots abs err ~0.02, fine at the 2e-2 gate).
  - exps: all four heads on Act ([128,1024] per head per pt, 1.03us
    each); Act and DVE then sit at ~4.15us/pt and ~4.17us/pt -- the
    balanced elementwise floor (a Schraudolph bit-trick exp on DVE is
    plumbed in via N_SCH but measures slower, so N_SCH=0).
  - divide: reciprocal_approx_fast over the whole po tile + two [32,1024]
    muls (psum x sbuf at offset partitions -- the only structure the BIR
    verifier allows; two-psum-input divide and >32-partition windows at
    base 32 are rejected by neuronxcc).
  - software pipeline: slot j emits A(j)/exps and C(j-5)+divides in a
    pa-ping-pong-aware interleave; q(j+4) woven in 2-pt units; v conv
    woven into slots 0-4 while C is deferred (CLAG=5), with the C-lag
    tapered away over the last ~10 slots so the tail stays short.
  - PSUM: pa 2x[128,1024] + po 2x[128,1024] = 8 banks; q-conv psums and
    the k/v conv accumulators borrow the same pools pre-stream.
"""

import numpy as np
import ml_dtypes

try:
    import concourse.bass as bass  # noqa: F401
except ImportError:  # pragma: no cover
    import sys
    sys.path.insert(0, "/opt/trn_rl_repo")

import concourse.bass as bass
import concourse.mybir as mybir
import concourse.tile as tile
from concourse import bacc
from concourse.bass_utils import run_bass_kernel_spmd

F32 = mybir.dt.float32
F32R = mybir.dt.float32r
BF16 = mybir.dt.bfloat16
I16 = mybir.dt.int16

B, DIM, H, W = 4, 256, 128, 128
INNER, P = 256, 8
HEADS, HG = 8, 4          # total heads, heads per group
C = INNER // HEADS        # 32 head channels
HWF = H * W               # 16384 flattened positions
KEYS = (H // P) * (W // P)  # 256 patches
N_CORES = 8
PT = 512                  # position tile
NPT = HWF // PT           # 32 position tiles

CLAG = 5                  # C stream lag behind A (v conv fills the gap)
SCH_A = 184.66502980931393  # 128/ln2 (bf16 schraudolph scale)
SCH_B = 16249.0             # 127*128 - 7 (calibrated offset)

_CACHE = {}
_EYE = np.ascontiguousarray(np.tile(np.eye(64, dtype=np.float32), (2, 1)))
BF = ml_dtypes.bfloat16


def _build():
    nc = bacc.Bacc(trn_type="TRN2", target_bir_lowering=False, debug=False)

    x_d = nc.dram_tensor("x", [DIM, HWF], BF16, kind="ExternalInput")
    wq_d = nc.dram_tensor("wq", [128, 256], BF16, kind="ExternalInput")
    wk_d = nc.dram_tensor("wk", [128, 16384], BF16, kind="ExternalInput")
    wv_d = nc.dram_tensor("wv", [128, 16384], BF16, kind="ExternalInput")
    # cols: 0 bq, 1 bk, 2 bv
    bias_d = nc.dram_tensor("bias", [128, 3], F32, kind="ExternalInput")
    eye_d = nc.dram_tensor("eye", [128, 64], F32, kind="ExternalInput")
    out_d = nc.dram_tensor("out", [128, HWF], BF16, kind="ExternalOutput")

    with tile.TileContext(nc) as tc:
        with tc.tile_pool(name="const", bufs=1) as cw, \
             tc.tile_pool(name="xq", bufs=1) as xq, \
             tc.tile_pool(name="wkv", bufs=6) as wkv, \
             tc.tile_pool(name="qt", bufs=13) as qtp, \
             tc.tile_pool(name="ep", bufs=28) as ep, \
             tc.tile_pool(name="e3p", bufs=2) as e3p, \
             tc.tile_pool(name="rp", bufs=6) as rp, \
             tc.tile_pool(name="op", bufs=4) as op, \
             tc.tile_pool(name="pa", bufs=2, space="PSUM") as pa, \
             tc.tile_pool(name="po", bufs=2, space="PSUM") as po:

            # ---- tiny consts (DMAs deferred until after the critical
            # x0/wk0 stream; first consumer is the q-copy at ~12us) ----
            wq_sb = cw.tile([128, 256], BF16)
            bias_sb = cw.tile([128, 3], F32)
            ident = cw.tile([128, 64], F32)

            k_ext = cw.tile([128, 512], BF16)
            nc.vector.memset(k_ext[:], 0.0)
            v_sb = cw.tile([128, KEYS], F32)
            vt_sb = cw.tile([128, 512], BF16)
            nc.vector.memset(vt_sb[:], 1.0)

            # PE p-state warm-up fodder
            warm_w = cw.tile([128, 64], BF16)
            nc.vector.memset(warm_w[:], 0.0)

            # ---- x / wk / wv DMAs over three queues, ordered so x-ck0 and
            # wk's first chunks land earliest (k conv ck0 can then start
            # ~8us in, while x-ck1 + the rest stream in behind) ----
            x_t = [xq.tile([128, HWF], BF16, tag=f"x{ck}", name=f"x{ck}")
                   for ck in range(2)]

            def x_dma(eng, ck, xc):
                cs = slice(xc * (HWF // 4), (xc + 1) * (HWF // 4))
                eng.dma_start(
                    x_t[ck][:, cs], x_d.ap()[ck * 128:(ck + 1) * 128, cs])

            wk_t, wv_t = [], []

            def w_dma(eng, lst, w_d, c_):
                nm = "wk" if lst is wk_t else "wv"
                wt = wkv.tile([128, 2048], BF16, tag="w", name=f"{nm}{c_}")
                eng.dma_start(wt[:], w_d.ap()[:, c_ * 2048:(c_ + 1) * 2048])
                lst.append(wt)

            # x as 16 half-chunks: all of ck0 first (3 queues round-robin)
            # so the k conv's ck0 half can start ~6.5us in, then ck1
            # interleaved with wk.
            def x_dma8(eng, ck, c8):
                cs = slice(c8 * (HWF // 8), (c8 + 1) * (HWF // 8))
                eng.dma_start(
                    x_t[ck][:, cs], x_d.ap()[ck * 128:(ck + 1) * 128, cs])

            q3 = [nc.sync, nc.scalar, nc.gpsimd]
            for c8 in range(8):
                x_dma8(q3[c8 % 3], 0, c8)
            w_dma(nc.gpsimd, wk_t, wk_d, 0)
            w_dma(nc.gpsimd, wk_t, wk_d, 1)
            nc.gpsimd.dma_start(wq_sb[:], wq_d.ap())
            nc.gpsimd.dma_start(bias_sb[:], bias_d.ap())
            nc.gpsimd.dma_start(ident[:], eye_d.ap())
            for c8 in range(8):
                x_dma8(q3[c8 % 3], 1, c8)
            warm_e = cw.tile([128, 32], BF16)
            nc.scalar.activation(warm_e[:], warm_w[:, 0:32],
                                 mybir.ActivationFunctionType.Exp)
            w_dma(nc.gpsimd, wk_t, wk_d, 2)
            w_dma(nc.gpsimd, wk_t, wk_d, 3)
            w_dma(nc.sync, wk_t, wk_d, 4)
            w_dma(nc.scalar, wk_t, wk_d, 5)
            w_dma(nc.sync, wk_t, wk_d, 6)
            w_dma(nc.scalar, wk_t, wk_d, 7)
            for c_ in range(8):
                w_dma(nc.gpsimd if c_ % 2 == 0 else nc.sync, wv_t, wv_d, c_)

            xv = [x_t[ck][:].rearrange(
                "p (ph i pw j) -> p i j ph pw", ph=16, i=8, pw=16, j=8)
                for ck in range(2)]

            def conv_chunk(wt, pk, ck, chunk, first, last):
                # 16 matmuls of one pre-loaded [128,2048] weight chunk
                for s in range(16):
                    ij = chunk * 16 + s
                    i_, j_ = ij // 8, ij % 8
                    nc.tensor.matmul(
                        pk[:, 0:KEYS],
                        wt[:, s * 128:(s + 1) * 128],
                        xv[ck][:, i_, j_],
                        start=(first and ij == 0),
                        stop=(last and ij == 63))

            # ---- PE warm-up: chained dummy matmuls during the x load ----
            warm_p = po.tile([128, 64], F32, tag="po", name="warm")
            for _ in range(90):
                nc.tensor.matmul(warm_p[0:64, 0:64], warm_w[:, 0:64],
                                 warm_w[:, 0:64], start=True, stop=True)

            # ---- k conv (first real PE work; gated on x-ck0 + wk chunks) ----
            pk_k = pa.tile([128, 1024], F32, tag="pa", name="pk_k")
            for ck in range(2):
                for chunk in range(4):
                    conv_chunk(wk_t[ck * 4 + chunk], pk_k, ck, chunk,
                               ck == 0 and chunk == 0, ck == 1 and chunk == 3)

            # ---- q conv (2-pt units) + bias copy (1 unit Act, rest DVE) ----
            q_t = {}

            def make_q2(jj, pool):
                # pts 2jj, 2jj+1 share one psum tile and one bias-copy
                pq = pool.tile([128, 1024], F32, tag=pool.name, name=f"pq{jj}")
                for half in range(2):
                    s2 = slice((2 * jj + half) * PT, (2 * jj + half + 1) * PT)
                    cs = slice(half * PT, (half + 1) * PT)
                    nc.tensor.matmul(pq[:, cs], wq_sb[:, 0:128],
                                     x_t[0][:, s2], start=True, stop=False)
                    nc.tensor.matmul(pq[:, cs], wq_sb[:, 128:256],
                                     x_t[1][:, s2], start=False, stop=True)
                q2 = qtp.tile([128, 1024], BF16, tag="q", name=f"qt{jj}")
                if jj % 3 == 0:
                    nc.scalar.activation(q2[:], pq[:],
                                         mybir.ActivationFunctionType.Identity,
                                         bias=bias_sb[:, 0:1])
                else:
                    nc.vector.tensor_scalar_add(q2[:], pq[:], bias_sb[:, 0:1])
                q_t[2 * jj] = q2[:, 0:PT]
                q_t[2 * jj + 1] = q2[:, PT:1024]

            # pre-stream q for the first 4 pts
            make_q2(0, po)
            make_q2(1, pa)

            # k_ext: block col (p*2+kb)*128 holds head 2hp+p's keys (kb
            # chunk) on rows hp*64+p*32..+32; adds split DVE/Act.
            for hp in range(2):
                for p in range(2):
                    for kb in range(2):
                        rows = slice(hp * 64 + p * 32, hp * 64 + p * 32 + 32)
                        cols = slice((p * 2 + kb) * 128, (p * 2 + kb) * 128 + 128)
                        if (p * 2 + kb) % 2 == 0:
                            nc.vector.tensor_scalar_add(
                                k_ext[rows, cols],
                                pk_k[rows, kb * 128:(kb + 1) * 128],
                                bias_sb[rows, 1:2])
                        else:
                            nc.scalar.activation(
                                k_ext[rows, cols],
                                pk_k[rows, kb * 128:(kb + 1) * 128],
                                mybir.ActivationFunctionType.Identity,
                                bias=bias_sb[rows, 1:2])

            # ---- stream helpers ----
            def emit_A(j, h, use_dve):
                hp, p = h // 2, h % 2
                qs = q_t[j][hp * 64:(hp + 1) * 64, :]
                pa_t = pa.tile([128, 1024], F32, tag="pa")
                nc.tensor.matmul(
                    pa_t[:, 0:512],
                    k_ext[hp * 64:(hp + 1) * 64,
                          (p * 2) * 128:(p * 2) * 128 + 128],
                    qs, start=True, stop=True)
                nc.tensor.matmul(
                    pa_t[:, 512:1024],
                    k_ext[hp * 64:(hp + 1) * 64,
                          (p * 2 + 1) * 128:(p * 2 + 1) * 128 + 128],
                    qs, start=True, stop=True)
                if h == HG - 1:
                    q_t[j] = None
                if use_dve:
                    # whole head via DVE schraudolph bits (bf16 exp approx)
                    e3 = e3p.tile([128, 1024], I16, tag="e3")
                    nc.vector.tensor_scalar(
                        out=e3[:], in0=pa_t[:],
                        scalar1=SCH_A, scalar2=SCH_B,
                        op0=mybir.AluOpType.mult, op1=mybir.AluOpType.add)
                    return e3[:].bitcast(BF16)
                e_t = ep.tile([128, 1024], BF16, tag="e")
                nc.scalar.activation(e_t[:], pa_t[:],
                                     mybir.ActivationFunctionType.Exp)
                return e_t[:]

            def emit_C(j, h, e_ap, po_t):
                # head h -> rows (h%2)*64..+64 (num +0..32, den +32..64),
                # cols (h//2)*512..+512
                rs = slice((h % 2) * 64, (h % 2) * 64 + 64)
                cs = slice((h // 2) * PT, (h // 2) * PT + PT)
                nc.tensor.matmul(po_t[rs, cs],
                                 vt_sb[:, (h * 2) * 64:(h * 2) * 64 + 64],
                                 e_ap[:, 0:512], start=True, stop=False)
                nc.tensor.matmul(po_t[rs, cs],
                                 vt_sb[:, (h * 2 + 1) * 64:(h * 2 + 1) * 64 + 64],
                                 e_ap[:, 512:1024], start=False, stop=True)

            def _out_dst(j, parity):
                s2 = slice(j * PT, (j + 1) * PT)
                return out_d.ap()[:, s2].rearrange(
                    "(pr two p) n -> two p pr n", pr=2, two=2, p=32)[parity]

            def emit_div(j, po_t):
                # one reciprocal over all 128 rows (num-row results unused;
                # cost is free-dim only), then one mul per head-parity
                rinv = rp.tile([128, 1024], F32, tag="r")
                nc.vector.reciprocal_approx_fast(rinv[:], po_t[:])
                oeven = op.tile([32, 1024], BF16, tag="o")
                nc.vector.tensor_mul(oeven[:], po_t[0:32, :], rinv[32:64, :])
                nc.sync.dma_start(_out_dst(j, 0), oeven[:])
                oodd = op.tile([32, 1024], BF16, tag="o", name="oodd")
                nc.vector.tensor_mul(oodd[:], po_t[64:96, :], rinv[96:128, :])
                nc.sync.dma_start(_out_dst(j, 1), oodd[:])

            # ---- software-pipelined stream ----
            e_live = {}
            po_live = {}
            c_cur = 0

            N_SCH = 0  # pts whose h3 exp goes to DVE (Act/DVE balance knob)

            def a_slot(j):
                use_dve = (j * N_SCH) // NPT != ((j + 1) * N_SCH) // NPT
                for h in range(HG):
                    e_live[(j, h)] = emit_A(j, h, use_dve and h == HG - 1)

            def c_slot(c):
                po_t = po.tile([128, 1024], F32, tag="po", name=f"po{c}")
                po_live[c] = po_t
                for h in range(HG):
                    emit_C(c, h, e_live.pop((c, h)), po_t)
                emit_div(c, po_live.pop(c))

            def ac_slot(j, cs):
                # A/C interleave tuned to the pa 2-buffer ping-pong: A0,A1
                # up front so Act's exp stream starts early in the slot, C
                # matmuls filling the windows where the next A would have
                # to wait for an exp to free its pa buffer.
                use_dve = (j * N_SCH) // NPT != ((j + 1) * N_SCH) // NPT
                for c in cs:
                    po_live[c] = po.tile([128, 1024], F32, tag="po",
                                         name=f"po{c}")
                order = ["a0", "a1", "c0", "c1", "a2", "c2", "a3", "c3"]
                for step in order:
                    h = int(step[1])
                    if step[0] == "a":
                        e_live[(j, h)] = emit_A(j, h, use_dve and h == HG - 1)
                    else:
                        for c in cs:
                            emit_C(c, h, e_live.pop((c, h)), po_live[c])
                for c in cs:
                    emit_div(c, po_live.pop(c))

            V_SLOT_CHUNKS = (2, 2, 2, 1, 1)  # v conv chunks per early slot
            NV = len(V_SLOT_CHUNKS)
            pk_v = None
            for j in range(NPT):
                # C stream (lags A; tapered at the end)
                c_target = j - CLAG
                if j >= NPT - 11:
                    c_target = min(c_target + (j - (NPT - 11)) // 2, j - 1)
                cs = list(range(c_cur, c_target + 1))
                c_cur = max(c_cur, c_target + 1)

                ac_slot(j, cs)

                # v conv woven into the pre-C slots
                if j < NV:
                    if j == 0:
                        pk_v = po.tile([128, 1024], F32, tag="po", name="pk_v")
                    base = sum(V_SLOT_CHUNKS[:j])
                    for c_ in range(base, base + V_SLOT_CHUNKS[j]):
                        ck, chunk = divmod(c_, 4)
                        conv_chunk(wv_t[c_], pk_v, ck, chunk,
                                   c_ == 0, c_ == 7)
                    if j == NV - 1:
                        # v epilogue: bias add, transposes, vt copies
                        nc.vector.tensor_scalar_add(
                            v_sb[:], pk_v[:, 0:KEYS], bias_sb[:, 2:3])
                        for hp in range(2):
                            for kb in range(2):
                                ptr = po.tile([128, 64], F32, tag="po",
                                              name="ptr")
                                nc.tensor.transpose(
                                    ptr[:],
                                    v_sb[hp * 64:(hp + 1) * 64,
                                         kb * 128:(kb + 1) * 128],
                                    ident[hp * 64:(hp + 1) * 64, :])
                                for p in range(2):
                                    u = (hp * 2 + p) * 2 + kb
                                    nc.vector.tensor_copy(
                                        vt_sb[:, u * 64:u * 64 + 32],
                                        ptr[:, p * 32:(p + 1) * 32])

                # q conv just-in-time, 4 pts ahead (2-pt units, even slots)
                if (j + 4) % 2 == 0 and j + 4 < NPT:
                    make_q2((j + 4) // 2, pa)

            while c_cur < NPT:
                c_slot(c_cur)
                c_cur += 1

    nc.compile()
    return nc


def _head_index(g):
    # device row m = h_local*32 + c_idx  ->  full-channel c_idx*8 + 4g + h_local
    m = np.arange(128)
    return (m % 32) * 8 + 4 * g + (m // 32)


def _prep_w_kv(Wf, idx):
    # [128o,256c,8,8] -> [128p, (ck, ij, m)] with lhsT layout per
    # (ck, chunk): wt[p, ij*128+m] = W[idx[m], ck*128+p, i, j]
    a = Wf[idx].reshape(128, 256, 64)          # [m, c, ij]
    a = a.transpose(1, 2, 0)                   # [c, ij, m]
    a = a.reshape(2, 128, 64, 128)             # [ck, p, ij, m]
    a = a.transpose(1, 0, 2, 3)                # [p, ck, ij, m]
    return np.ascontiguousarray(a.reshape(128, 16384).astype(BF))


def kernel(x, Wq, bq, Wk, bk, Wv, bv):
    if "nc" not in _CACHE:
        _CACHE["nc"] = _build()
    nc = _CACHE["nc"]

    x = np.asarray(x, np.float32)
    Wq = np.asarray(Wq, np.float32)
    bq = np.asarray(bq, np.float32)
    Wk = np.asarray(Wk, np.float32)
    bk = np.asarray(bk, np.float32)
    Wv = np.asarray(Wv, np.float32)
    bv = np.asarray(bv, np.float32)
    in_maps = []
    idxs = []
    for g in range(2):
        idx = _head_index(g)
        idxs.append(idx)
        # wq[p, ck*128+m] = Wq[idx[m], ck*128+p]
        wq_h = np.ascontiguousarray(
            Wq[idx, :, 0, 0].T.reshape(2, 128, 128).transpose(1, 0, 2)
            .reshape(128, 256).astype(BF))
        wk_h = _prep_w_kv(Wk, idx)
        wv_h = _prep_w_kv(Wv, idx)
        bias_h = np.ascontiguousarray(
            np.stack([bq[idx], bk[idx], bv[idx]], axis=1), np.float32)
        for b in range(B):
            in_maps.append({
                "x": np.ascontiguousarray(x[b].reshape(DIM, HWF).astype(BF)),
                "wq": wq_h, "wk": wk_h, "wv": wv_h,
                "bias": bias_h, "eye": _EYE,
            })
    # core order: core = b*2 + g  -> reorder in_maps built as g-major
    order = [g * B + b for b in range(B) for g in range(2)]
    in_maps = [in_maps[i] for i in order]

    res = run_bass_kernel_spmd(nc, in_maps, core_ids=list(range(N_CORES)))
    _CACHE["last"] = res

    out = np.empty((B, INNER, H, W), np.float32)
    for core in range(N_CORES):
        b, g = core // 2, core % 2
        out[b, idxs[g]] = np.asarray(
            res.results[core]["out"], np.float32).reshape(128, H, W)
    return out
